# revision 9
# baseline (speedup 1.0000x reference)
"""Trainium2 Bass kernel for nn_BoundarySeg (gnn_message_passing).

Computation (per example b, position j, MAX_SPAN_LEN=6 window):
    first[j]  = sum_{d=0..5, j+d<L} w[j, j+d] * h[j+d]
    second[j] = h[j] * sum_{d, j+d<L} w[j, j+d]
    out[j]    = concat([first, second])            # [B, L, 2H]

Only the 6-diagonal band of the [B, L, L] adjacency is ever used, so the
host extracts that band (a pure strided gather / data-layout step) and
builds small banded weight matrices; all arithmetic (the windowed weighted
sums and the scaled copy) runs on-device.

Device strategy (pure data parallel, B=16 sharded 2-per-core over 8 cores):
  - `first` as a banded matmul on the PE array: for each 128-row output
    tile, lhsT_a [128,128] holds the in-tile band diagonals and a tiny
    lhsT_b [5,128] holds the seam contributions from the next h tile;
    both accumulate into one PSUM tile.
  - `second` as a per-partition tensor_scalar multiply on the Vector
    engine, with the window sums reduced on-device from the band.
  - All per-example inputs (h, banded weights, band) are packed into one
    contiguous DRAM tensor and loaded with a single DMA per example, so
    consumers carry at most one DMA semaphore wait (PE Matmult supports
    only one sync wait slot).
  - HBM traffic per core ~21 MiB (in 8.3 MiB + out 12.6 MiB): memory-bound.
"""

import os
import sys

import numpy as np

if "/opt/trn_rl_repo" not in sys.path:
    sys.path.insert(0, "/opt/trn_rl_repo")

B, L, H = 16, 1024, 768
D = 6             # MAX_SPAN_LEN
NCORES = 8
BP = B // NCORES  # examples per core
P = 128
NT = L // P       # 128-row tiles per example

# column offsets (in f32 elements) inside the packed per-example SBUF tile
H_OFF = 0
WA_OFF = H_OFF + NT * H           # 6144
WB_OFF = WA_OFF + NT * P          # 7168
BAND_OFF = WB_OFF + (NT - 1) * P  # 8064
F_TOT = BAND_OFF + NT * D         # 8112

_nc_cache = None


def _build_bass():
    import concourse.tile as tile
    from concourse import bacc, mybir

    f32 = mybir.dt.float32
    nc = bacc.Bacc("TRN2", target_bir_lowering=False)

    mega_d = nc.dram_tensor("mega", [BP, P, F_TOT], f32, kind="ExternalInput")
    out_d = nc.dram_tensor("out", [BP, L, 2 * H], f32, kind="ExternalOutput")

    with tile.TileContext(nc) as tc:
        with (
            tc.tile_pool(name="mpool", bufs=2) as mpool,
            tc.tile_pool(name="opool", bufs=4) as opool,
            tc.tile_pool(name="spool", bufs=4) as spool,
            tc.tile_pool(name="pspool", bufs=3, space="PSUM") as pspool,
        ):
            for ex in range(BP):
                mega = mpool.tile([P, F_TOT], f32)
                nc.sync.dma_start(out=mega, in_=mega_d[ex])

                for t in range(NT):
                    psum = pspool.tile([P, H], f32)
                    lhsTa = mega[:, WA_OFF + t * P : WA_OFF + (t + 1) * P]
                    rhs = mega[:, t * H : (t + 1) * H]
                    last = t == NT - 1
                    # fp32 matmul: moving operand <= 512 cols (one PSUM bank)
                    for c0, c1 in ((0, 512), (512, H)):
                        nc.tensor.matmul(
                            out=psum[:, c0:c1],
                            lhsT=lhsTa,
                            rhs=rhs[:, c0:c1],
                            start=True,
                            stop=last,
                        )
                    if not last:
                        # seam: windows of rows 128t+123..127 spill into the
                        # first 5 rows of the next h tile
                        lhsTb = mega[0:5, WB_OFF + t * P : WB_OFF + (t + 1) * P]
                        rhs_b = mega[0:5, (t + 1) * H : (t + 2) * H]
                        for c0, c1 in ((0, 512), (512, H)):
                            nc.tensor.matmul(
                                out=psum[:, c0:c1],
                                lhsT=lhsTb,
                                rhs=rhs_b[:, c0:c1],
                                start=False,
                                stop=True,
                            )
                    out_sb = opool.tile([P, 2 * H], f32)
                    nc.scalar.copy(out=out_sb[:, 0:H], in_=psum[:])
                    wsum = spool.tile([P, 1], f32)
                    nc.vector.reduce_sum(
                        out=wsum,
                        in_=mega[:, BAND_OFF + t * D : BAND_OFF + (t + 1) * D],
                        axis=mybir.AxisListType.X,
                    )
                    nc.vector.tensor_scalar_mul(
                        out=out_sb[:, H : 2 * H], in0=rhs, scalar1=wsum
                    )
                    nc.sync.dma_start(
                        out=out_d[ex, t * P : (t + 1) * P, :], in_=out_sb
                    )
    nc.compile()
    return nc


def _host_prep(span_adjacency, bound_hidden):
    """Extract the used 6-wide diagonal band and pack h + banded matmul
    operands into one contiguous tensor per example. Pure gather/layout —
    no arithmetic on the data."""
    adj = span_adjacency.reshape(B, L, L)
    band = np.zeros((B, L, D), dtype=np.float32)
    for d in range(D):
        # band[b, j, d] = adj[b, j, j+d] for j+d < L, else 0
        band[:, : L - d, d] = np.diagonal(adj, offset=d, axis1=1, axis2=2)
    band_t = band.reshape(B, NT, P, D)

    # lhsT_a[b, t, k, m] = band[b, 128t+m, k-m] for 0 <= k-m <= 5
    wa = np.zeros((B, NT, P, P), dtype=np.float32)
    for d in range(D):
        m = np.arange(P - d)
        wa[:, :, m + d, m] = band_t[:, :, : P - d, d]

    # lhsT_b[b, t, k, m] = band[b, 128t+m, 128+k-m] for m >= 123+k (seam)
    wb = np.zeros((B, NT - 1, 5, P), dtype=np.float32)
    for k in range(5):
        for m in range(123 + k, P):
            wb[:, :, k, m] = band_t[:, : NT - 1, m, P + k - m]

    mega = np.zeros((B, P, F_TOT), dtype=np.float32)
    # h block: [p, t*H + c] = h[128t+p, c]
    mega[:, :, H_OFF:WA_OFF] = (
        bound_hidden.reshape(B, NT, P, H).transpose(0, 2, 1, 3).reshape(B, P, NT * H)
    )
    # wa block: [k, t*P + m]
    mega[:, :, WA_OFF:WB_OFF] = wa.transpose(0, 2, 1, 3).reshape(B, P, NT * P)
    # wb block: [k, t*P + m], only partitions 0..4 carry data
    mega[:, 0:5, WB_OFF:BAND_OFF] = wb.transpose(0, 2, 1, 3).reshape(B, 5, (NT - 1) * P)
    # band block: [p, t*D + d]
    mega[:, :, BAND_OFF:F_TOT] = band_t.transpose(0, 2, 1, 3).reshape(B, P, NT * D)

    return [
        {"mega": np.ascontiguousarray(mega[BP * c : BP * (c + 1)])}
        for c in range(NCORES)
    ]


def run(span_adjacency, bound_hidden, trace=False):
    """Run on 8 NeuronCores; returns (out [B, L, 2H] f32, exec_time_ns|None)."""
    global _nc_cache
    from concourse import bass_utils

    in_maps = _host_prep(np.asarray(span_adjacency), np.asarray(bound_hidden))
    if _nc_cache is None:
        _nc_cache = _build_bass()
    res = bass_utils.run_bass_kernel_spmd(
        _nc_cache, in_maps, core_ids=list(range(NCORES)), trace=trace
    )
    out = np.concatenate([r["out"] for r in res.results], axis=0)
    return out, res.exec_time_ns


def kernel(span_adjacency, bound_hidden):
    out, _ = run(span_adjacency, bound_hidden, trace=False)
    return out
